# revision 42
# baseline (speedup 1.0000x reference)
"""Trainium2 Bass kernel for nn_AttentionBlock: 8-core data-parallel over batch.

Reference computation (per batch b):
  cx = X[b] @ Wx^T               [K,R]   (K=49 regions, R=49, H=1024)
  ch = h_t[b] @ Wh^T             [T,R]   (T=128)
  z[t,k] = sum_r Wa[r] * tanh(cx[k,r] + ch[t,r])
  alpha = softmax_k(z)           [T,K]
  out[b] = alpha @ X[b]          [T,H]

Design (per core, 16 batches, processed in groups of GB=4):
  - k-slot mapping: k = 28*J + s (J in {0,1}); zT row(k) = 64*J + s.
    X rows are cast-DMA'd (SWDGE f32->bf16) pre-scattered into xb_all at
    rows 64*J + s; gap rows zeroed once (split DVE/GpSimd memsets).
  - h: one SWDGE cast-DMA per group, then ONE xbar DMA transpose
    [128, GB*1024] -> [128, GB, HT, 128] per group (the xbar's col-major
    row mapping r -> (partition r%128, slot r//128) lands h-chunks exactly
    on partitions). Same trick for xT from xb_all. No PE transposes or
    DVE casts anywhere in the steady state.
  - ch mirrored to partitions 0:49 & 64:113 by a [WhT|0|WhT] stationary,
    computed as GB-wide quad matmuls; cx per group via 16 col-tiled
    matmuls (tile_position (0,0)/(0,64)) reading xT with a strided k-AP.
  - S = tanh(ch + cx): one DVE TT (PSUM ch source, broadcast APs, q<QD)
    plus one GpSimd TT (SBUF ch copy, q>=QD); single ScalarE tanh.
  - z: 28 col-tiled accumulating matmuls, Wa slabs at col offsets 2rr;
    zT lands [64J+s, t] in one PSUM bank; gap rows written zero by the
    zero slab columns.
  - softmax: no max-shift (|z| <= ||Wa||_1 < 1 by construction); exp
    PSUM->SBUF bf16 = unnormalized alphaT; denom via ones-column matmul
    (zeroed at pad rows); reciprocal; normalization fused into the
    PSUM->SBUF output copy via per-partition tensor_scalar (DVE) and
    activation-Copy-with-scale (ScalarE) split at OSPLIT.
  - Group g+1's DMAs (casts + transposes) are issued before group g's
    batch work so the sync-queue transposes never sit behind out-DMAs.
"""

import sys

sys.path.insert(0, "/opt/trn_rl_repo")

import numpy as np

import concourse.bass as bass
import concourse.bacc as bacc
import concourse.tile as tile
from concourse import mybir
from concourse.bass_utils import run_bass_kernel_spmd
from concourse.masks import make_identity

B, T, K, H = 128, 128, 49, 1024
R = 49
NCORES = 8
BL = B // NCORES  # batches per core
GB = 4  # batches per group
NG = BL // GB
HT = H // 128
NQ = 28  # q slots (14 per J-half)
NQV = 25  # valid q slots
KR = 92  # rows of the scattered k layout (64 + 28)
QD = 12  # q slots added on DVE; rest (NQV-QD) on GpSimd
OSPLIT = 850  # out cols normalized on DVE; rest on ScalarE
F32 = mybir.dt.float32
BF16 = mybir.dt.bfloat16

_CACHE = {}


def _ap(base, off, dims):
    return bass.AP(tensor=base.tensor, offset=base.offset + off, ap=dims)


def build():
    nc = bacc.Bacc("TRN2", target_bir_lowering=False, debug=False, num_devices=NCORES)

    X_d = nc.dram_tensor("X", [BL, K, H], F32, kind="ExternalInput").ap()
    ht_d = nc.dram_tensor("h_t", [BL, T, H], F32, kind="ExternalInput").ap()
    Wx_d = nc.dram_tensor("Wx", [R, H], F32, kind="ExternalInput").ap()
    Wh_d = nc.dram_tensor("Wh", [R, H], F32, kind="ExternalInput").ap()
    Wa_d = nc.dram_tensor("Wa", [1, R], F32, kind="ExternalInput").ap()
    out_d = nc.dram_tensor("out", [BL, T, H], F32, kind="ExternalOutput").ap()

    with tile.TileContext(nc) as tc:
        with (
            tc.tile_pool(name="consts", bufs=1) as consts,
            tc.tile_pool(name="xall", bufs=1) as xall,
            tc.tile_pool(name="hbp", bufs=2) as hb_pool,
            tc.tile_pool(name="hTp", bufs=3) as hT_pool,
            tc.tile_pool(name="chp", bufs=3) as ch_pool,
            tc.tile_pool(name="sp", bufs=3) as s_pool,
            tc.tile_pool(name="ap", bufs=4) as a_pool,
            tc.tile_pool(name="rp", bufs=4) as r_pool,
            tc.tile_pool(name="ob", bufs=4) as o_pool,
            tc.tile_pool(name="pcc", bufs=2, space="PSUM") as pcc,
            tc.tile_pool(name="psZ", bufs=2, space="PSUM") as psZ,
            tc.tile_pool(name="psO", bufs=1, space="PSUM") as psO,
            tc.tile_pool(name="pset", bufs=1, space="PSUM") as pset,
        ):
            # ================= setup =================
            # X tile first: gap rows must be zero; split the big memset
            xb_all = xall.tile([96, BL, H], BF16)
            nc.vector.memset(xb_all[:, 0 : BL // 2, :], 0.0)
            nc.gpsimd.memset(xb_all[:, BL // 2 : BL, :], 0.0)

            ident = consts.tile([128, 128], F32)
            make_identity(nc, ident[:])
            warm = consts.tile([1, 2], F32)
            nc.scalar.activation(
                warm[:], ident[0:1, 0:2], mybir.ActivationFunctionType.Tanh
            )

            # weights: natural f32 load, PE transpose, mirrored bf16 copies.
            # Wh transposes rotate through the psZ pool (2 bufs), Wx through pset.
            wn = consts.tile([R, 2 * H], F32)
            nc.scalar.dma_start(out=wn[:, 0:H], in_=_ap(Wh_d, 0, [[H, R], [1, H]]))
            nc.scalar.dma_start(out=wn[:, H : 2 * H], in_=_ap(Wx_d, 0, [[H, R], [1, H]]))
            WhT2 = consts.tile([128, HT, 128], BF16)  # [p, j, 0:49|pad|64:113]
            WxT = consts.tile([128, HT, 64], BF16)
            nc.vector.memset(WhT2[:], 0.0)
            nc.vector.memset(WxT[:], 0.0)
            stile = pset.tile([128, 448], F32, tag="setup")
            for j in range(HT):
                tpx = stile[:, j * 56 : j * 56 + R]
                nc.tensor.transpose(
                    tpx, wn[:, H + j * 128 : H + (j + 1) * 128], ident[0:R, 0:R]
                )
                nc.vector.tensor_copy(WxT[:, j, 0:R], tpx)
            for j in range(HT):
                zt = psZ.tile([128, 132], F32, tag="z")
                tph = zt[:, 0:R]
                nc.tensor.transpose(
                    tph, wn[:, j * 128 : (j + 1) * 128], ident[0:R, 0:R]
                )
                nc.vector.tensor_copy(WhT2[:, j, 0:R], tph)
                nc.vector.tensor_copy(WhT2[:, j, 64 : 64 + R], tph)

            # Wa onto partitions, bf16
            waf = consts.tile([R, 1], F32)
            nc.scalar.dma_start(out=waf[:], in_=_ap(Wa_d, 0, [[1, R], [1, 1]]))
            wab = consts.tile([R, 1], BF16)
            nc.vector.tensor_copy(wab[:], waf[:])
            # z slabs: [113, 14, 64]; col 2rr <- Wa at rows 0:49, col 2rr+1 <- rows 64:113
            slab = consts.tile([128, 14, 64], BF16)
            nc.vector.memset(slab[:], 0.0)
            for rr in range(14):
                nc.vector.tensor_copy(slab[0:R, rr, 2 * rr : 2 * rr + 1], wab[:])
                nc.vector.tensor_copy(
                    slab[64 : 64 + R, rr, 2 * rr + 1 : 2 * rr + 2], wab[:]
                )

            # ones column for the denominator matmul (valid k rows only)
            onescol = consts.tile([KR, 1], BF16)
            nc.vector.memset(onescol[:], 0.0)
            nc.vector.memset(onescol[0:28, :], 1.0)
            nc.vector.memset(onescol[64 : 64 + (K - 28), :], 1.0)

            def _resh(apobj, dims):
                return bass.AP(
                    tensor=apobj.tensor, offset=apobj.offset, ap=[apobj.ap[0]] + dims
                )

            # ================= main loop =================
            def emit_cast(g):
                g0 = g * GB
                nc.gpsimd.dma_start(
                    out=xb_all[0:28, g0 : g0 + GB, :],
                    in_=_ap(X_d, g0 * K * H, [[H, 28], [K * H, GB], [1, H]]),
                )
                nc.gpsimd.dma_start(
                    out=xb_all[64 : 64 + (K - 28), g0 : g0 + GB, :],
                    in_=_ap(X_d, g0 * K * H + 28 * H, [[H, K - 28], [K * H, GB], [1, H]]),
                )
                hb = hb_pool.tile([128, GB, H], BF16, tag="hb")
                nc.gpsimd.dma_start(
                    out=hb[:],
                    in_=_ap(ht_d, g0 * T * H, [[H, T], [T * H, GB], [1, H]]),
                )
                return hb

            def emit_xt(g):
                g0 = g * GB
                xTg = hT_pool.tile([128, GB, HT, 96], BF16, tag="xT")
                nc.sync.dma_start(
                    out=xTg[:],
                    in_=xb_all[0:96, g0 : g0 + GB, :],
                    transpose=True,
                )
                return xTg

            def emit_ht(hb, bb):
                hTb = hT_pool.tile([128, HT, 128], BF16, tag="hT")
                nc.sync.dma_start(out=hTb[:], in_=hb[:, bb, :], transpose=True)
                return hTb

            def emit_group_pre(g, xTg):
                # cx: cxg[p, b', q]; rows 0:49 even-k, 64:113 odd-k
                cxps = pset.tile([128, 448], F32, tag="setup")
                if g < 1:
                    nc.vector.memset(cxps[:], 0.0)
                xt = xTg[:]
                for par in range(2):
                    dst = _resh(
                        cxps[64 * par : 64 * par + R, :], [[NQ, GB], [1, NQ]]
                    )
                    for j in range(HT):
                        nc.tensor.matmul(
                            dst,
                            WxT[:, j, 0:R],
                            _ap(
                                xt,
                                j * 96 + par,
                                [xt.ap[0], [HT * 96, GB], [64, 2], [2, 14]],
                            ),
                            start=(j == 0),
                            stop=(j == HT - 1),
                            tile_position=(0, 64 * par),
                        )
                cxg = ch_pool.tile([128, GB, NQ], BF16, tag="cxg")
                nc.vector.tensor_copy(
                    cxg[0:113, :, :], _resh(cxps[0:113, :], [[NQ, GB], [1, NQ]])
                )
                return cxg

            def emit_early(b, bb, cxg, hTb):
                cc1 = pcc.tile([113, 128], F32, tag="cc1")
                for j in range(HT):
                    nc.tensor.matmul(
                        cc1[:],
                        WhT2[:, j, 0:113],
                        hTb[:, j, :],
                        start=(j == 0),
                        stop=(j == HT - 1),
                    )
                chsb = ch_pool.tile([113, 128], BF16, tag="chsb")
                nc.vector.tensor_copy(chsb[:], cc1[:])
                # S = tanh(ch + cx), bf16 [113, 25, 128]
                S = s_pool.tile([128, NQ, 128], BF16, tag="S")
                if b < 3:
                    nc.vector.memset(S[:, NQV:NQ, :], 0.0)
                c1 = cc1[:]
                ca = cxg[:]
                cs = chsb[:]
                nc.vector.tensor_add(
                    S[0:113, 0:QD, :],
                    _ap(c1, 0, [[c1.ap[0][0], 113], [0, QD], [1, 128]]),
                    _ap(ca, bb * NQ, [[ca.ap[0][0], 113], [1, QD], [0, 128]]),
                )
                nc.gpsimd.tensor_tensor(
                    S[0:113, QD:NQV, :],
                    _ap(cs, 0, [[cs.ap[0][0], 113], [0, NQV - QD], [1, 128]]),
                    _ap(
                        ca,
                        bb * NQ + QD,
                        [[ca.ap[0][0], 113], [1, NQV - QD], [0, 128]],
                    ),
                    mybir.AluOpType.add,
                )
                nc.scalar.activation(
                    S[0:113, 0:NQV, :],
                    S[0:113, 0:NQV, :],
                    mybir.ActivationFunctionType.Tanh,
                )
                # zT[64J+s, t] via col-tiled accumulating matmuls
                zal = psZ.tile([128, 132], F32, tag="z")
                for rr in range(14):
                    for J in range(2):
                        nc.tensor.matmul(
                            zal[64 * J : 64 * J + 64, 0:128],
                            slab[0:113, rr, :],
                            S[0:113, 14 * J + rr, :],
                            start=(rr == 0),
                            stop=(rr == 13),
                            tile_position=(0, 64 * J),
                        )
                return zal

            def emit_late1(b, zal):
                alphaT = a_pool.tile([KR, 128], BF16, tag="alphaT")
                nc.scalar.activation(
                    alphaT[:], zal[0:KR, 0:128], mybir.ActivationFunctionType.Exp
                )
                dps = zal[:, 128:129]
                nc.tensor.matmul(dps, alphaT[:], onescol[:], start=True, stop=True)
                rden = r_pool.tile([128, 1], F32, tag="rden")
                nc.vector.reciprocal(rden[:], dps)
                ob = psO.tile([128, H], F32, tag="ob")
                for half in range(2):
                    nc.tensor.matmul(
                        ob[:, half * 512 : (half + 1) * 512],
                        alphaT[:],
                        xb_all[0:KR, b, half * 512 : (half + 1) * 512],
                        start=True,
                        stop=True,
                    )
                return rden, ob

            def emit_late2(b, rden, ob):
                osb = o_pool.tile([128, H], F32, tag="osb")
                nc.vector.tensor_scalar(
                    osb[:, 0:OSPLIT],
                    ob[:, 0:OSPLIT],
                    rden[:],
                    None,
                    mybir.AluOpType.mult,
                )
                if OSPLIT < H:
                    nc.scalar.activation(
                        osb[:, OSPLIT:H],
                        ob[:, OSPLIT:H],
                        mybir.ActivationFunctionType.Copy,
                        scale=rden[:],
                    )
                nc.sync.dma_start(
                    out=_ap(out_d, b * T * H, [[H, T], [1, H]]), in_=osb[:]
                )

            hbs = {0: emit_cast(0)}
            xTg = emit_xt(0)
            hTb_next = emit_ht(hbs[0], 0)
            for g in range(NG):
                if g + 1 < NG:
                    hbs[g + 1] = emit_cast(g + 1)
                    nxT = emit_xt(g + 1)
                cxg = emit_group_pre(g, xTg)
                for bb in range(GB):
                    b = g * GB + bb
                    hTb = hTb_next
                    # issue the next batch's transpose one batch ahead
                    nb = b + 1
                    if nb < BL:
                        hTb_next = emit_ht(hbs[nb // GB], nb % GB)
                    zal = emit_early(b, bb, cxg, hTb)
                    rden, ob = emit_late1(b, zal)
                    emit_late2(b, rden, ob)
                if g + 1 < NG:
                    xTg = nxT

    nc.compile()
    return nc


def _get_nc():
    if "nc" not in _CACHE:
        _CACHE["nc"] = build()
    return _CACHE["nc"]


def kernel(X, h_t, Wx, Wh, Wa):
    nc = _get_nc()
    X = np.ascontiguousarray(X, dtype=np.float32)
    h_t = np.ascontiguousarray(h_t, dtype=np.float32)
    Wx = np.ascontiguousarray(Wx, dtype=np.float32)
    Wh = np.ascontiguousarray(Wh, dtype=np.float32)
    Wa = np.ascontiguousarray(Wa, dtype=np.float32)
    in_maps = [
        {
            "X": X[c * BL : (c + 1) * BL],
            "h_t": h_t[c * BL : (c + 1) * BL],
            "Wx": Wx,
            "Wh": Wh,
            "Wa": Wa,
        }
        for c in range(NCORES)
    ]
    res = run_bass_kernel_spmd(nc, in_maps, core_ids=list(range(NCORES)))
    return np.concatenate([res.results[c]["out"] for c in range(NCORES)], axis=0)


# revision 43
# speedup vs baseline: 1.0903x; 1.0903x over previous
"""Trainium2 Bass kernel for nn_AttentionBlock: 8-core data-parallel over batch.

Reference computation (per batch b):
  cx = X[b] @ Wx^T               [K,R]   (K=49 regions, R=49, H=1024)
  ch = h_t[b] @ Wh^T             [T,R]   (T=128)
  z[t,k] = sum_r Wa[r] * tanh(cx[k,r] + ch[t,r])
  alpha = softmax_k(z)           [T,K]
  out[b] = alpha @ X[b]          [T,H]

Design (per core, 16 batches, processed in groups of GB=4):
  - k-slot mapping: k = 28*J + s (J in {0,1}); zT row(k) = 64*J + s.
    X rows are cast-DMA'd (SWDGE f32->bf16) pre-scattered into xb_all at
    rows 64*J + s; gap rows zeroed once (split DVE/GpSimd memsets).
  - h: one SWDGE cast-DMA per group, then ONE xbar DMA transpose
    [128, GB*1024] -> [128, GB, HT, 128] per group (the xbar's col-major
    row mapping r -> (partition r%128, slot r//128) lands h-chunks exactly
    on partitions). Same trick for xT from xb_all. No PE transposes or
    DVE casts anywhere in the steady state.
  - ch mirrored to partitions 0:49 & 64:113 by a [WhT|0|WhT] stationary,
    computed as GB-wide quad matmuls; cx per group via 16 col-tiled
    matmuls (tile_position (0,0)/(0,64)) reading xT with a strided k-AP.
  - S = tanh(ch + cx): one DVE TT (PSUM ch source, broadcast APs, q<QD)
    plus one GpSimd TT (SBUF ch copy, q>=QD); single ScalarE tanh.
  - z: 28 col-tiled accumulating matmuls, Wa slabs at col offsets 2rr;
    zT lands [64J+s, t] in one PSUM bank; gap rows written zero by the
    zero slab columns.
  - softmax: no max-shift (|z| <= ||Wa||_1 < 1 by construction); exp
    PSUM->SBUF bf16 = unnormalized alphaT; denom via ones-column matmul
    (zeroed at pad rows); reciprocal; normalization fused into the
    PSUM->SBUF output copy via per-partition tensor_scalar (DVE) and
    activation-Copy-with-scale (ScalarE) split at OSPLIT.
  - Group g+1's DMAs (casts + transposes) are issued before group g's
    batch work so the sync-queue transposes never sit behind out-DMAs.
"""

import sys

sys.path.insert(0, "/opt/trn_rl_repo")

import numpy as np

import concourse.bass as bass
import concourse.bacc as bacc
import concourse.tile as tile
from concourse import mybir
from concourse.bass_utils import run_bass_kernel_spmd
from concourse.masks import make_identity

B, T, K, H = 128, 128, 49, 1024
R = 49
NCORES = 8
BL = B // NCORES  # batches per core
GB = 4  # batches per group
NG = BL // GB
HT = H // 128
NQ = 28  # q slots (14 per J-half)
NQV = 25  # valid q slots
KR = 92  # rows of the scattered k layout (64 + 28)
QD = 12  # q slots added on DVE; rest (NQV-QD) on GpSimd
OSPLIT = 850  # out cols normalized on DVE; rest on ScalarE
F32 = mybir.dt.float32
BF16 = mybir.dt.bfloat16

_CACHE = {}


def _ap(base, off, dims):
    return bass.AP(tensor=base.tensor, offset=base.offset + off, ap=dims)


def build():
    nc = bacc.Bacc("TRN2", target_bir_lowering=False, debug=False, num_devices=NCORES)

    X_d = nc.dram_tensor("X", [BL, K, H], F32, kind="ExternalInput").ap()
    ht_d = nc.dram_tensor("h_t", [BL, T, H], F32, kind="ExternalInput").ap()
    Wx_d = nc.dram_tensor("Wx", [R, H], F32, kind="ExternalInput").ap()
    Wh_d = nc.dram_tensor("Wh", [R, H], F32, kind="ExternalInput").ap()
    Wa_d = nc.dram_tensor("Wa", [1, R], F32, kind="ExternalInput").ap()
    out_d = nc.dram_tensor("out", [BL, T, H], F32, kind="ExternalOutput").ap()

    with tile.TileContext(nc) as tc:
        with (
            tc.tile_pool(name="consts", bufs=1) as consts,
            tc.tile_pool(name="xall", bufs=1) as xall,
            tc.tile_pool(name="hbp", bufs=2) as hb_pool,
            tc.tile_pool(name="hTp", bufs=2) as hT_pool,
            tc.tile_pool(name="chp", bufs=2) as ch_pool,
            tc.tile_pool(name="sp", bufs=3) as s_pool,
            tc.tile_pool(name="ap", bufs=4) as a_pool,
            tc.tile_pool(name="rp", bufs=4) as r_pool,
            tc.tile_pool(name="ob", bufs=4) as o_pool,
            tc.tile_pool(name="pcc", bufs=2, space="PSUM") as pcc,
            tc.tile_pool(name="psZ", bufs=2, space="PSUM") as psZ,
            tc.tile_pool(name="psO", bufs=1, space="PSUM") as psO,
            tc.tile_pool(name="pset", bufs=1, space="PSUM") as pset,
        ):
            # ================= setup =================
            # X tile first: gap rows must be zero; split the big memset
            xb_all = xall.tile([96, BL, H], BF16)
            nc.vector.memset(xb_all[:, 0 : BL // 2, :], 0.0)
            nc.gpsimd.memset(xb_all[:, BL // 2 : BL, :], 0.0)

            ident = consts.tile([128, 128], F32)
            make_identity(nc, ident[:])
            warm = consts.tile([1, 2], F32)
            nc.scalar.activation(
                warm[:], ident[0:1, 0:2], mybir.ActivationFunctionType.Tanh
            )

            # weights: natural f32 load, PE transpose, mirrored bf16 copies.
            # Wh transposes rotate through the psZ pool (2 bufs), Wx through pset.
            wn = consts.tile([R, 2 * H], F32)
            nc.scalar.dma_start(out=wn[:, 0:H], in_=_ap(Wh_d, 0, [[H, R], [1, H]]))
            nc.scalar.dma_start(out=wn[:, H : 2 * H], in_=_ap(Wx_d, 0, [[H, R], [1, H]]))
            WhT2 = consts.tile([128, HT, 128], BF16)  # [p, j, 0:49|pad|64:113]
            WxT = consts.tile([128, HT, 64], BF16)
            nc.vector.memset(WhT2[:], 0.0)
            nc.vector.memset(WxT[:], 0.0)
            stile = pset.tile([128, 448], F32, tag="setup")
            for j in range(HT):
                tpx = stile[:, j * 56 : j * 56 + R]
                nc.tensor.transpose(
                    tpx, wn[:, H + j * 128 : H + (j + 1) * 128], ident[0:R, 0:R]
                )
                nc.vector.tensor_copy(WxT[:, j, 0:R], tpx)
            for j in range(HT):
                zt = psZ.tile([128, 132], F32, tag="z")
                tph = zt[:, 0:R]
                nc.tensor.transpose(
                    tph, wn[:, j * 128 : (j + 1) * 128], ident[0:R, 0:R]
                )
                nc.vector.tensor_copy(WhT2[:, j, 0:R], tph)
                nc.vector.tensor_copy(WhT2[:, j, 64 : 64 + R], tph)

            # Wa onto partitions, bf16
            waf = consts.tile([R, 1], F32)
            nc.scalar.dma_start(out=waf[:], in_=_ap(Wa_d, 0, [[1, R], [1, 1]]))
            wab = consts.tile([R, 1], BF16)
            nc.vector.tensor_copy(wab[:], waf[:])
            # z slabs: [113, 14, 64]; col 2rr <- Wa at rows 0:49, col 2rr+1 <- rows 64:113
            slab = consts.tile([128, 14, 64], BF16)
            nc.vector.memset(slab[:], 0.0)
            for rr in range(14):
                nc.vector.tensor_copy(slab[0:R, rr, 2 * rr : 2 * rr + 1], wab[:])
                nc.vector.tensor_copy(
                    slab[64 : 64 + R, rr, 2 * rr + 1 : 2 * rr + 2], wab[:]
                )

            # ones column for the denominator matmul (valid k rows only)
            onescol = consts.tile([KR, 1], BF16)
            nc.vector.memset(onescol[:], 0.0)
            nc.vector.memset(onescol[0:28, :], 1.0)
            nc.vector.memset(onescol[64 : 64 + (K - 28), :], 1.0)

            def _resh(apobj, dims):
                return bass.AP(
                    tensor=apobj.tensor, offset=apobj.offset, ap=[apobj.ap[0]] + dims
                )

            # ================= main loop =================
            def emit_cast(g):
                g0 = g * GB
                nc.gpsimd.dma_start(
                    out=xb_all[0:28, g0 : g0 + GB, :],
                    in_=_ap(X_d, g0 * K * H, [[H, 28], [K * H, GB], [1, H]]),
                )
                nc.gpsimd.dma_start(
                    out=xb_all[64 : 64 + (K - 28), g0 : g0 + GB, :],
                    in_=_ap(X_d, g0 * K * H + 28 * H, [[H, K - 28], [K * H, GB], [1, H]]),
                )
                hb = hb_pool.tile([128, GB, H], BF16, tag="hb")
                nc.gpsimd.dma_start(
                    out=hb[:],
                    in_=_ap(ht_d, g0 * T * H, [[H, T], [T * H, GB], [1, H]]),
                )
                return hb

            def emit_transposes(g, hb):
                g0 = g * GB
                xTg = hT_pool.tile([128, GB, HT, 96], BF16, tag="xT")
                nc.sync.dma_start(
                    out=xTg[:],
                    in_=xb_all[0:96, g0 : g0 + GB, :],
                    transpose=True,
                )
                hTt = hT_pool.tile([128, GB, HT, 128], BF16, tag="hT")
                nc.sync.dma_start(out=hTt[:], in_=hb[:], transpose=True)
                return hTt, xTg

            def emit_group_pre(g, hTt, xTg):
                # cx: cxg[p, b', q]; rows 0:49 even-k, 64:113 odd-k
                cxps = pset.tile([128, 448], F32, tag="setup")
                if g < 1:
                    nc.vector.memset(cxps[:], 0.0)
                xt = xTg[:]
                for par in range(2):
                    dst = _resh(
                        cxps[64 * par : 64 * par + R, :], [[NQ, GB], [1, NQ]]
                    )
                    for j in range(HT):
                        nc.tensor.matmul(
                            dst,
                            WxT[:, j, 0:R],
                            _ap(
                                xt,
                                j * 96 + par,
                                [xt.ap[0], [HT * 96, GB], [64, 2], [2, 14]],
                            ),
                            start=(j == 0),
                            stop=(j == HT - 1),
                            tile_position=(0, 64 * par),
                        )
                cxg = ch_pool.tile([128, GB, NQ], BF16, tag="cxg")
                nc.vector.tensor_copy(
                    cxg[0:113, :, :], _resh(cxps[0:113, :], [[NQ, GB], [1, NQ]])
                )
                cc1 = pcc.tile([113, GB, 128], F32, tag="cc1")
                for j in range(HT):
                    nc.tensor.matmul(
                        cc1[:],
                        WhT2[:, j, 0:113],
                        hTt[:, :, j, :],
                        start=(j == 0),
                        stop=(j == HT - 1),
                    )
                chsb = ch_pool.tile([113, GB, 128], BF16, tag="chsb")
                nc.vector.tensor_copy(chsb[:], cc1[:])
                return cxg, cc1, chsb

            def emit_early(b, bb, cxg, cc1, chsb):
                # S = tanh(ch + cx), bf16 [113, 25, 128]
                S = s_pool.tile([128, NQ, 128], BF16, tag="S")
                if b < 3:
                    nc.vector.memset(S[:, NQV:NQ, :], 0.0)
                c1 = cc1[:]
                ca = cxg[:]
                cs = chsb[:]
                nc.vector.tensor_add(
                    S[0:113, 0:QD, :],
                    _ap(c1, bb * 128, [[c1.ap[0][0], 113], [0, QD], [1, 128]]),
                    _ap(ca, bb * NQ, [[ca.ap[0][0], 113], [1, QD], [0, 128]]),
                )
                nc.gpsimd.tensor_tensor(
                    S[0:113, QD:NQV, :],
                    _ap(
                        cs, bb * 128, [[cs.ap[0][0], 113], [0, NQV - QD], [1, 128]]
                    ),
                    _ap(
                        ca,
                        bb * NQ + QD,
                        [[ca.ap[0][0], 113], [1, NQV - QD], [0, 128]],
                    ),
                    mybir.AluOpType.add,
                )
                nc.scalar.activation(
                    S[0:113, 0:NQV, :],
                    S[0:113, 0:NQV, :],
                    mybir.ActivationFunctionType.Tanh,
                )
                # zT[64J+s, t] via col-tiled accumulating matmuls
                zal = psZ.tile([128, 132], F32, tag="z")
                for rr in range(14):
                    for J in range(2):
                        nc.tensor.matmul(
                            zal[64 * J : 64 * J + 64, 0:128],
                            slab[0:113, rr, :],
                            S[0:113, 14 * J + rr, :],
                            start=(rr == 0),
                            stop=(rr == 13),
                            tile_position=(0, 64 * J),
                        )
                return zal

            def emit_late1(b, zal):
                alphaT = a_pool.tile([KR, 128], BF16, tag="alphaT")
                nc.scalar.activation(
                    alphaT[:], zal[0:KR, 0:128], mybir.ActivationFunctionType.Exp
                )
                dps = zal[:, 128:129]
                nc.tensor.matmul(dps, alphaT[:], onescol[:], start=True, stop=True)
                rden = r_pool.tile([128, 1], F32, tag="rden")
                nc.vector.reciprocal(rden[:], dps)
                ob = psO.tile([128, H], F32, tag="ob")
                for half in range(2):
                    nc.tensor.matmul(
                        ob[:, half * 512 : (half + 1) * 512],
                        alphaT[:],
                        xb_all[0:KR, b, half * 512 : (half + 1) * 512],
                        start=True,
                        stop=True,
                    )
                return rden, ob

            def emit_late2(b, rden, ob):
                osb = o_pool.tile([128, H], F32, tag="osb")
                nc.vector.tensor_scalar(
                    osb[:, 0:OSPLIT],
                    ob[:, 0:OSPLIT],
                    rden[:],
                    None,
                    mybir.AluOpType.mult,
                )
                if OSPLIT < H:
                    nc.scalar.activation(
                        osb[:, OSPLIT:H],
                        ob[:, OSPLIT:H],
                        mybir.ActivationFunctionType.Copy,
                        scale=rden[:],
                    )
                nc.sync.dma_start(
                    out=_ap(out_d, b * T * H, [[H, T], [1, H]]), in_=osb[:]
                )

            hb0 = emit_cast(0)
            fetchT = emit_transposes(0, hb0)
            for g in range(NG):
                hTt, xTg = fetchT
                if g + 1 < NG:
                    nextC = emit_cast(g + 1)
                    fetchT = emit_transposes(g + 1, nextC)
                cxg, cc1, chsb = emit_group_pre(g, hTt, xTg)
                for bb in range(GB):
                    b = g * GB + bb
                    zal = emit_early(b, bb, cxg, cc1, chsb)
                    rden, ob = emit_late1(b, zal)
                    emit_late2(b, rden, ob)

    nc.compile()
    return nc


def _get_nc():
    if "nc" not in _CACHE:
        _CACHE["nc"] = build()
    return _CACHE["nc"]


def kernel(X, h_t, Wx, Wh, Wa):
    nc = _get_nc()
    X = np.ascontiguousarray(X, dtype=np.float32)
    h_t = np.ascontiguousarray(h_t, dtype=np.float32)
    Wx = np.ascontiguousarray(Wx, dtype=np.float32)
    Wh = np.ascontiguousarray(Wh, dtype=np.float32)
    Wa = np.ascontiguousarray(Wa, dtype=np.float32)
    in_maps = [
        {
            "X": X[c * BL : (c + 1) * BL],
            "h_t": h_t[c * BL : (c + 1) * BL],
            "Wx": Wx,
            "Wh": Wh,
            "Wa": Wa,
        }
        for c in range(NCORES)
    ]
    res = run_bass_kernel_spmd(nc, in_maps, core_ids=list(range(NCORES)))
    return np.concatenate([res.results[c]["out"] for c in range(NCORES)], axis=0)
